# revision 1
# baseline (speedup 1.0000x reference)
"""BiLSTM (B=16, T=2048, D=U=256) on 8 TRN2 NeuronCores.

Sharding: 8 cores = 2 directions x 4 batch-shards (B_local=4 per core).
Backward cores receive x time-reversed on the host; all cores run the same
SPMD program (a forward scan), so no collectives are needed.  Keras-style
go_backwards semantics mean the backward half is emitted in iteration
order, which is exactly the scan order on the backward cores.

Per-core kernel: precompute xw[t] = x_t @ W on the TensorEngine (bf16,
gate order [cand i f o], candidate columns pre-doubled on the host so
tanh(x) = 2*sigmoid(2x)-1 needs only the sigmoid table), then run the
2048-step recurrence.  Per step the PE accumulates 16 R-tile matmuls
(R and h in float8e4 — halves the per-step LDWEIGHTS stream; validated
rel err ~5e-3) on top of identity-matmuls that inject xw_t into PSUM,
with the [cand,i,f] and [o] gates in separate PSUM banks so ScalarE's
batched sigmoid over [cand|i|f] can start while the o-chunk matmuls
finish.  VectorE does the cell update (cand affine, one fused
[i|f]*[cand|c] multiply, pair add); tanh(sigmoid(s)) is approximated as
K*sigmoid(AL*s + BE) (max err 8.6e-4) so the output nonlinearity is a
single ScalarE op with K folded into R and the output copy, and
c' = sigmoid(s) runs off the critical path.  h is written twice: fp8 for
the recurrence, bf16 for the staged f32 output.
"""

import numpy as np

F32 = None  # set on first build

_CACHE = {}

T = 2048
B = 16
D = 256
U = 256
G = 4 * U
BL = 4  # batch per core

K_PHI = 0.7589144336406901
AL_PHI = 1.0834263081088795
BE_PHI = 0.44379053813456204


def _patch_tile_drain():
    """This container's walrus accepts only one sem-wait/update per
    instruction; spread Tile's final-drain waits across NOPs."""
    import concourse.tile as tile
    import concourse.mybir as mybir
    from concourse.vector_clock import ScopedClock

    if getattr(tile.TileContext, "_lstm_patched", False):
        return

    def _drain_and_barrier(self, tick_clock, wait_clock):
        carrier = self.nc.sync.nop(nofuse=True, hint="final_wait_carrier")
        wait_clock.add_sem_waits(
            carrier.ins, ScopedClock({None: tick_clock.global_clock})
        )
        si = carrier.ins.sync_info
        waits = list(si.on_wait or []) if si is not None else []
        if len(waits) > 1:
            si.on_wait = waits[:1]
            for wx in waits[1:]:
                n = self.nc.sync.nop(nofuse=True, hint="final_wait_extra")
                if n.ins.sync_info is None:
                    n.ins.sync_info = mybir.SyncInfo(on_wait=[wx], on_update=[])
                else:
                    n.ins.sync_info.on_wait = [wx]
        self.nc.sync.drain()
        self.nc.all_engine_barrier()
        assert self.sems is not None
        popped = self.nc._tile_sem_poison_stack.pop()
        assert popped is self._sem_poison
        self.nc.clear_and_free_semaphores(list(self.sems.allocated().values()))
        self.nc.all_engine_barrier()

    tile.TileContext._drain_and_barrier = _drain_and_barrier
    tile.TileContext._lstm_patched = True


def _split_syncs(nc, max_waits=1, max_updates=1):
    import concourse.mybir as mybir

    ctr = [0]

    def mknop(engine, waits, updates):
        ctr[0] += 1
        return mybir.InstNoOp(
            name=f"syncfix-{ctr[0]}",
            engine=engine,
            sync_info=mybir.SyncInfo(on_wait=list(waits), on_update=list(updates)),
        )

    for f in nc.m.functions:
        for bb in f.blocks:
            changed = False
            out = []
            for inst in bb.instructions:
                si = inst.sync_info
                if si is None or inst.engine == mybir.EngineType.Unassigned:
                    out.append(inst)
                    continue
                waits = list(si.on_wait or [])
                updates = list(si.on_update or [])
                if len(waits) <= max_waits and len(updates) <= max_updates:
                    out.append(inst)
                    continue
                changed = True
                for wx in waits[:-max_waits] if max_waits else waits:
                    out.append(mknop(inst.engine, [wx], []))
                si.on_wait = waits[-max_waits:] if max_waits else []
                extra_u = updates[max_updates:] if max_updates else updates
                si.on_update = updates[:max_updates] if max_updates else []
                out.append(inst)
                for ux in extra_u:
                    out.append(mknop(inst.engine, [], [ux]))
            if changed:
                bb.instructions = out
    return nc


def _build_v3(seg=128, proj_tb=128, split_sig=True, fp8=True, use_tanh=False,
              hfull=True, B=BL):
    import concourse.bass as bass
    import concourse.mybir as mybir
    import concourse.tile as tile
    from contextlib import ExitStack

    _patch_tile_drain()
    F32 = mybir.dt.float32
    BF16 = mybir.dt.bfloat16
    FP8 = mybir.dt.float8e4
    SIG = mybir.ActivationFunctionType.Sigmoid
    COPY = mybir.ActivationFunctionType.Copy
    nc = bass.Bass()
    xt = nc.dram_tensor("xt", [2, 128, T * B], F32, kind="ExternalInput")
    w = nc.dram_tensor("w", [D, G], F32, kind="ExternalInput")
    r = nc.dram_tensor("r", [U, G], F32, kind="ExternalInput")
    bcg = nc.dram_tensor("bcg", [128, 2], F32, kind="ExternalInput")
    out = nc.dram_tensor("out", [2, 128, T * B], F32, kind="ExternalOutput")

    RDT = FP8 if fp8 else BF16
    NB = B
    HW = 2 * NB
    W8 = 8 * NB

    with ExitStack() as ctx:
        tc = ctx.enter_context(tile.TileContext(nc))
        const = ctx.enter_context(tc.tile_pool(name="const", bufs=1))
        big = ctx.enter_context(tc.tile_pool(name="big", bufs=1))
        wstage = ctx.enter_context(tc.tile_pool(name="wstage", bufs=2))
        xload = ctx.enter_context(tc.tile_pool(name="xload", bufs=2))
        xcast = ctx.enter_context(tc.tile_pool(name="xcast", bufs=2))
        ppsum = ctx.enter_context(tc.tile_pool(name="ppsum", bufs=2, space="PSUM"))
        gpsum = ctx.enter_context(tc.tile_pool(name="gpsum", bufs=2, space="PSUM"))
        work = ctx.enter_context(tc.tile_pool(name="work", bufs=3))
        hsegp = ctx.enter_context(tc.tile_pool(name="hsegp", bufs=2))
        ostage = ctx.enter_context(tc.tile_pool(name="ostage", bufs=2))

        wb = const.tile([128, 2, G], BF16)
        rb = const.tile([128, 2, G], RDT)
        bct = const.tile([128, 2], F32)
        ident = const.tile([128, 128], BF16)
        hzero = const.tile([128, HW], RDT)
        bphi = const.tile([128, 1], F32)
        nc.vector.memset(bphi[:, :], BE_PHI)

        for src, dst in ((w, wb), (r, rb)):
            for k in range(2):
                st = wstage.tile([128, G], F32, tag="wst")
                nc.sync.dma_start(out=st[:, :], in_=src[k * 128:(k + 1) * 128, :])
                nc.scalar.copy(dst[:, k, :], st[:, :])
        nc.sync.dma_start(out=bct[:, :], in_=bcg[:, :])
        from concourse.masks import make_identity
        make_identity(nc, ident[:, :])
        nc.vector.memset(hzero[:, :], 0.0)

        xw = big.tile([128, T, W8], BF16)
        if hfull:
            h2 = big.tile([128, T, HW], RDT)
        else:
            h2 = big.tile([128, 2, HW], RDT)  # fp8 h/K ping-pong

        ntb = T // proj_tb
        ntok = proj_tb * B
        for tb in range(ntb):
            t0 = tb * proj_tb
            xf = xload.tile([128, 2, ntok], F32)
            xb = xcast.tile([128, 2, ntok], BF16)
            for k in range(2):
                nc.sync.dma_start(
                    out=xf[:, k, :], in_=xt[k, :, t0 * B:(t0 + proj_tb) * B],
                )
            nc.scalar.copy(xb[:, :, :], xf[:, :, :])
            for c in range(8):
                ps = ppsum.tile([128, ntok], F32)
                for k in range(2):
                    nc.tensor.matmul(
                        ps[:, :],
                        wb[:, k, c * 128:(c + 1) * 128],
                        xb[:, k, :],
                        start=(k == 0),
                        stop=(k == 1),
                    )
                dst = xw[:, t0:t0 + proj_tb, c * NB:(c + 1) * NB]
                if c < 2:  # cand chunks carry the bias
                    nc.vector.tensor_scalar(
                        dst, ps[:, :], bct[:, c:c + 1], None,
                        mybir.AluOpType.add,
                    )
                elif c % 2 == 0:
                    nc.scalar.copy(dst, ps[:, :])
                else:
                    nc.vector.tensor_copy(dst, ps[:, :])

        # state: cand (0:HW) | c (HW:2HW)
        state = const.tile([128, 2 * HW], F32)
        nc.vector.memset(state[:, :], 0.0)
        nseg = T // seg
        for si in range(nseg):
            hseg = hsegp.tile([128, seg, HW], BF16)
            for tl in range(seg):
                t = si * seg + tl
                g = gpsum.tile([128, 6 * NB], F32, tag="gcif")
                go = gpsum.tile([128, 2 * NB], F32, tag="go")
                nc.tensor.matmul(
                    g[:, :], ident[:, :], xw[:, t, :6 * NB],
                    start=True, stop=False, skip_group_check=True,
                )
                nc.tensor.matmul(
                    go[:, :], ident[:, :], xw[:, t, 6 * NB:],
                    start=True, stop=False, skip_group_check=True,
                )

                def rmm(c, k, stop=False):
                    rhs = (hzero[:, k * NB:(k + 1) * NB] if t == 0
                           else h2[:, (t - 1) if hfull else (t - 1) % 2, k * NB:(k + 1) * NB])
                    dst = (g[:, c * NB:(c + 1) * NB] if c < 6
                           else go[:, (c - 6) * NB:(c - 5) * NB])
                    nc.tensor.matmul(
                        dst,
                        rb[:, k, c * 128:(c + 1) * 128],
                        rhs,
                        start=False, stop=stop, skip_group_check=True,
                    )

                u = work.tile([128, W8], F32, tag="u")
                for c in range(6):
                    for k in range(2):
                        rmm(c, k)
                TANH = mybir.ActivationFunctionType.Tanh
                if split_sig:
                    if use_tanh:
                        # cand = tanh(g_c) straight into state[:, 0:HW]
                        nc.scalar.activation(state[:, :HW], g[:, :2 * NB],
                                             TANH, scale=0.5)
                        nc.scalar.activation(u[:, HW:6 * NB], g[:, 2 * NB:], SIG)
                    else:
                        nc.scalar.activation(u[:, :6 * NB], g[:, :], SIG)
                for c in (6, 7):
                    for k in range(2):
                        rmm(c, k, stop=(c == 7 and k == 1))
                if split_sig:
                    nc.scalar.activation(u[:, 6 * NB:], go[:, :], SIG)
                else:
                    nc.scalar.activation(u[:, :6 * NB], g[:, :], SIG)
                    nc.scalar.activation(u[:, 6 * NB:], go[:, :], SIG)
                if not use_tanh:
                    # cand = 2*u_c - 1 -> state[:, 0:HW]
                    nc.vector.tensor_scalar(
                        state[:, :HW], u[:, :HW], 2.0, -1.0,
                        mybir.AluOpType.mult, mybir.AluOpType.add,
                    )
                # prod = [i|f] * [cand|c] -> [m2|m1]
                prod = work.tile([128, 4 * HW], F32, tag="prod")
                nc.vector.tensor_mul(
                    prod[:, :2 * HW], u[:, HW:3 * HW], state[:, :],
                )
                s = prod[:, 2 * HW:3 * HW]
                nc.vector.tensor_add(s, prod[:, :HW], prod[:, HW:2 * HW])
                phi = prod[:, 3 * HW:]
                nc.scalar.activation(phi, s, SIG, bias=bphi[:, :], scale=AL_PHI)
                nc.vector.tensor_mul(
                    h2[:, t if hfull else t % 2, :], phi, u[:, 6 * NB:],
                )
                nc.scalar.activation(state[:, HW:], s, SIG)
                nc.vector.tensor_mul(
                    hseg[:, tl, :], phi, u[:, 6 * NB:],
                )

            t0 = si * seg
            ost = ostage.tile([128, 2, seg, NB], F32)
            nc.scalar.activation(
                ost[:, :, :, :],
                hseg[:, :, :].rearrange("p t (k b) -> p k t b", k=2),
                COPY, scale=K_PHI,
            )
            for k in range(2):
                nc.sync.dma_start(
                    out=out[k, :, t0 * B:(t0 + seg) * B],
                    in_=ost[:, k, :, :],
                )
    _split_syncs(nc)
    return nc


def _prep_weights(Wd, Rd, bcd):
    # reference gate order [i f o c] -> kernel order [c i f o]
    perm = np.concatenate([
        np.arange(3 * U, 4 * U), np.arange(0, U),
        np.arange(U, 2 * U), np.arange(2 * U, 3 * U),
    ])
    Wp = np.ascontiguousarray(Wd[:, perm]).astype(np.float32)
    Rp = np.ascontiguousarray(Rd[:, perm]).astype(np.float32)
    Wp[:, :U] *= 2.0
    Rp[:, :U] *= 2.0
    Rp *= K_PHI
    bcg = np.ascontiguousarray((2.0 * bcd).reshape(2, 128).T).astype(np.float32)
    return Wp, Rp, bcg


def kernel(x, W_f, R_f, bc_f, W_b, R_b, bc_b):
    from concourse.bass_utils import run_bass_kernel_spmd

    x = np.asarray(x, dtype=np.float32)
    if "nc" not in _CACHE:
        _CACHE["nc"] = _build_v3()
    nc = _CACHE["nc"]

    Wf, Rf, bcgf = _prep_weights(np.asarray(W_f, np.float32),
                                 np.asarray(R_f, np.float32),
                                 np.asarray(bc_f, np.float32))
    Wb_, Rb_, bcgb = _prep_weights(np.asarray(W_b, np.float32),
                                   np.asarray(R_b, np.float32),
                                   np.asarray(bc_b, np.float32))

    in_maps = []
    for core in range(8):
        fwd = core < 4
        b0 = (core % 4) * BL
        xs = x[b0:b0 + BL]
        if not fwd:
            xs = xs[:, ::-1, :]
        xtr = np.ascontiguousarray(xs.transpose(2, 1, 0)).reshape(2, 128, T * BL)
        in_maps.append({
            "xt": xtr,
            "w": Wf if fwd else Wb_,
            "r": Rf if fwd else Rb_,
            "bcg": bcgf if fwd else bcgb,
        })

    res = run_bass_kernel_spmd(nc, in_maps, core_ids=list(range(8)))

    outp = np.empty((B, T, 2 * U), dtype=np.float32)
    for core in range(8):
        od = res.results[core]["out"]  # [2, 128, T*BL]
        hb = od.reshape(256, T, BL).transpose(2, 1, 0)  # [BL, T, U]
        b0 = (core % 4) * BL
        if core < 4:
            outp[b0:b0 + BL, :, 0:U] = hb
        else:
            outp[b0:b0 + BL, :, U:2 * U] = hb
    return outp



# revision 16
# speedup vs baseline: 15.7727x; 15.7727x over previous
"""BiLSTM (B=16, T=2048, D=U=256) on 8 TRN2 NeuronCores — time-sharded.

Sharding: 8 cores = 2 directions x 4 time-quarters.  Each core runs the
FULL batch (16) over its 512-step quarter, split into C=2 interleaved
chains x m=4 lane-groups (NB = 64 lanes per step) of 64 steps each plus
an 8-step warmup.  The cell c' = sigmoid(f*c + i*cand) contracts state
at >=4x per step, so an 8-step burn-in from zero state reaches the f32
noise floor (measured 3e-8); chunk boundaries are exact to tolerance.
Per core only 72 sequential steps run instead of 2048 — the recurrence
is latency-bound, so this is the dominant win.

Per step: x-projection runs just-in-time as fp8 DoubleRow matmuls
(K=256 contraction in one instruction) straight into the gate PSUM
bank; R matmuls (fp8 DoubleRow, K_PHI pre-folded, cand pre-doubled so
tanh comes from the sigmoid table) accumulate on top.  One sigmoid over
all four gates (bf16), DVE computes cand affine + i*cand + f*c + s and
AL*s+BE, a second sigmoid yields [c' | phi] in one instruction, and h
is written twice (fp8 for the recurrence rhs, bf16 for the staged
output).  Two chains interleave so Act/DVE stay busy while the other
chain's recurrence latency drains.  Output DMAs bf16; the host applies
K_PHI and widens to f32.
"""

import numpy as np

_CACHE = {}

T = 2048
B = 16
D = 256
U = 256
G = 4 * U

# time-sharding parameters
M_GRP = 4           # lane-groups (time sub-chunks) per chain
N_CH = 2            # chains per core
L_OUT = 512 // (N_CH * M_GRP)   # output steps per lane-group (64)
W_UP = 8            # warmup steps
TL = L_OUT + W_UP   # 72 steps per chain
NB = 16 * M_GRP     # 64 lanes per chain-step
HW = 2 * NB         # h width (2 U-halves x NB)
XBLK = 8            # steps per x-DMA block
SEG = 24            # steps per output DMA segment

USE_BIAS = True
DBG = None

K_PHI = 0.7589144336406901
AL_PHI = 1.0834263081088795
BE_PHI = 0.44379053813456204


def _patch_tile_drain():
    """This container's walrus accepts only one sem-wait/update per
    instruction; spread Tile's final-drain waits across NOPs."""
    import concourse.tile as tile
    import concourse.mybir as mybir
    from concourse.vector_clock import ScopedClock

    if getattr(tile.TileContext, "_lstm_patched", False):
        return

    def _drain_and_barrier(self, tick_clock, wait_clock):
        carrier = self.nc.sync.nop(nofuse=True, hint="final_wait_carrier")
        wait_clock.add_sem_waits(
            carrier.ins, ScopedClock({None: tick_clock.global_clock})
        )
        si = carrier.ins.sync_info
        waits = list(si.on_wait or []) if si is not None else []
        if len(waits) > 1:
            si.on_wait = waits[:1]
            for wx in waits[1:]:
                n = self.nc.sync.nop(nofuse=True, hint="final_wait_extra")
                if n.ins.sync_info is None:
                    n.ins.sync_info = mybir.SyncInfo(on_wait=[wx], on_update=[])
                else:
                    n.ins.sync_info.on_wait = [wx]
        self.nc.sync.drain()
        self.nc.all_engine_barrier()
        assert self.sems is not None
        popped = self.nc._tile_sem_poison_stack.pop()
        assert popped is self._sem_poison
        self.nc.clear_and_free_semaphores(list(self.sems.allocated().values()))
        self.nc.all_engine_barrier()

    tile.TileContext._drain_and_barrier = _drain_and_barrier
    tile.TileContext._lstm_patched = True


def _split_syncs(nc, max_waits=1, max_updates=1):
    import concourse.mybir as mybir

    ctr = [0]

    def mknop(engine, waits, updates):
        ctr[0] += 1
        return mybir.InstNoOp(
            name=f"syncfix-{ctr[0]}",
            engine=engine,
            sync_info=mybir.SyncInfo(on_wait=list(waits), on_update=list(updates)),
        )

    for f in nc.m.functions:
        for bb in f.blocks:
            changed = False
            out = []
            for inst in bb.instructions:
                si = inst.sync_info
                if si is None or inst.engine == mybir.EngineType.Unassigned:
                    out.append(inst)
                    continue
                waits = list(si.on_wait or [])
                updates = list(si.on_update or [])
                if len(waits) <= max_waits and len(updates) <= max_updates:
                    out.append(inst)
                    continue
                changed = True
                for wx in waits[:-max_waits] if max_waits else waits:
                    out.append(mknop(inst.engine, [wx], []))
                si.on_wait = waits[-max_waits:] if max_waits else []
                extra_u = updates[max_updates:] if max_updates else updates
                si.on_update = updates[:max_updates] if max_updates else []
                out.append(inst)
                for ux in extra_u:
                    out.append(mknop(inst.engine, [], [ux]))
            if changed:
                bb.instructions = out
    return nc


def _build_v4(fp8_proj=True, fp8_r=True):
    import concourse.bass as bass
    import concourse.mybir as mybir
    import concourse.tile as tile
    from contextlib import ExitStack

    _patch_tile_drain()
    F32 = mybir.dt.float32
    BF16 = mybir.dt.bfloat16
    FP8 = mybir.dt.float8e4
    SIG = mybir.ActivationFunctionType.Sigmoid
    DR = mybir.MatmulPerfMode.DoubleRow
    MULT = mybir.AluOpType.mult
    ADD = mybir.AluOpType.add

    XDT = FP8 if fp8_proj else BF16
    RDT = FP8 if fp8_r else BF16
    nc = bass.Bass()
    xt = nc.dram_tensor("xt", [N_CH, 2, 128, TL * NB], XDT, kind="ExternalInput")
    wd = nc.dram_tensor("wd", [128, 2, G], XDT, kind="ExternalInput")
    rd = nc.dram_tensor("rd", [128, 2, G], RDT, kind="ExternalInput")
    bcd = nc.dram_tensor("bcd", [1, 2 * 128], BF16, kind="ExternalInput")
    outd = nc.dram_tensor("outd", [N_CH, 2, 128, TL * NB], BF16,
                          kind="ExternalOutput")

    NPOS = TL
    NBLK = TL // XBLK

    with ExitStack() as ctx:
        tc = ctx.enter_context(tile.TileContext(nc))
        const = ctx.enter_context(tc.tile_pool(name="const", bufs=1))
        big = ctx.enter_context(tc.tile_pool(name="big", bufs=1))
        xpool = ctx.enter_context(tc.tile_pool(name="xpool", bufs=2))
        gpool = ctx.enter_context(tc.tile_pool(name="gpool", bufs=3, space="PSUM"))
        upool = ctx.enter_context(tc.tile_pool(name="upool", bufs=2))
        wpool = ctx.enter_context(tc.tile_pool(name="wpool", bufs=2))

        wt = const.tile([128, 2, G], XDT)
        rt = const.tile([128, 2, G], RDT)
        bct = const.tile([1, 2 * 128], BF16)
        ones = const.tile([1, NB], BF16)
        hz8 = const.tile([128, 2, NB], RDT)
        zsout = const.tile([128, 2, HW], BF16)

        nc.sync.dma_start(out=wt[:, :, :], in_=wd[:, :, :])
        nc.sync.dma_start(out=rt[:, :, :], in_=rd[:, :, :])
        nc.sync.dma_start(out=bct[:, :], in_=bcd[:, :])
        nc.vector.memset(ones[:, :], 1.0)
        nc.vector.memset(hz8[:, :, :], 0.0)
        nc.vector.memset(zsout[:, :, :], 0.0)

        # per-chain long-lived state
        hall = [big.tile([128, 2, TL, NB], BF16, tag=f"hall{c}",
                         name=f"hall{c}") for c in range(N_CH)]
        h8t = [big.tile([128, TL, 2, NB], FP8, tag=f"h8t{c}",
                        name=f"h8t{c}") for c in range(N_CH)]

        xbuf = [[None] * NBLK for _ in range(N_CH)]
        gtl = [[None] * NPOS for _ in range(N_CH)]
        ut = [[None] * NPOS for _ in range(N_CH)]
        soutt = [[None] * NPOS for _ in range(N_CH)]

        def dma_xblk(c, b):
            xb = xpool.tile([128, 2, XBLK * NB], XDT, tag=f"x{c}")
            xbuf[c][b] = xb
            for k in range(2):
                nc.sync.dma_start(
                    out=xb[:, k, :],
                    in_=xt[c, k, :, b * XBLK * NB:(b + 1) * XBLK * NB],
                )

        def proj(c, t):
            """JIT projection for step t into a fresh PSUM bank."""
            g = gpool.tile([128, 8 * NB], F32, tag=f"g{c}")
            gtl[c][t] = g
            xb = xbuf[c][t // XBLK]
            to = (t % XBLK) * NB
            # start=True only on the first matmul into the bank: start marks
            # the WHOLE bank pending-zero, so later-region first-writes clear
            # to zero on first touch.
            if fp8_proj:
                rhs = xb[:, :, to:to + NB]
                for cc in range(8):
                    nc.tensor.matmul(
                        g[:, cc * NB:(cc + 1) * NB],
                        wt[:, :, cc * 128:(cc + 1) * 128],
                        rhs,
                        start=(cc == 0), stop=False, perf_mode=DR,
                        skip_group_check=True,
                    )
            else:
                for cc in range(8):
                    for k in range(2):
                        nc.tensor.matmul(
                            g[:, cc * NB:(cc + 1) * NB],
                            wt[:, k, cc * 128:(cc + 1) * 128],
                            xb[:, k, to:to + NB],
                            start=(cc == 0 and k == 0), stop=False,
                            skip_group_check=True,
                        )
            # candidate bias (2*bc, zero in practice) via rank-1 matmul
            if USE_BIAS:
                for hh in range(2):
                    nc.tensor.matmul(
                        g[:, hh * NB:(hh + 1) * NB],
                        bct[:, hh * 128:(hh + 1) * 128],
                        ones[:, :],
                        start=False, stop=False, skip_group_check=True,
                    )

        def rstep(c, t):
            """R matmuls for step t + gate sigmoid."""
            g = gtl[c][t]
            if fp8_r:
                rhs = hz8[:, :, :] if t == 0 else h8t[c][:, t - 1, :, :]
                for cc in range(8):
                    nc.tensor.matmul(
                        g[:, cc * NB:(cc + 1) * NB],
                        rt[:, :, cc * 128:(cc + 1) * 128],
                        rhs,
                        start=False, stop=(cc == 7), perf_mode=DR,
                        skip_group_check=True,
                    )
            else:
                for cc in range(8):
                    for k in range(2):
                        rhs = (hz8[:, k, :] if t == 0
                               else hall[c][:, k, t - 1, :])
                        nc.tensor.matmul(
                            g[:, cc * NB:(cc + 1) * NB],
                            rt[:, k, cc * 128:(cc + 1) * 128],
                            rhs,
                            start=False, stop=(cc == 7 and k == 1),
                            skip_group_check=True,
                        )
            u = upool.tile([128, 8 * NB], BF16, tag=f"u{c}")
            ut[c][t] = u
            nc.scalar.activation(u[:, :], g[:, :], SIG)

        def dve_pre(c, t):
            """cand affine, products, s, AL*s+BE, then [c'|phi] sigmoid."""
            u = ut[c][t]
            cand = wpool.tile([128, HW], BF16, tag=f"cand{c}")
            pt = wpool.tile([128, 2, HW], BF16, tag=f"pt{c}")
            sin = wpool.tile([128, 2, HW], BF16, tag=f"sin{c}")
            sout = wpool.tile([128, 2, HW], BF16, tag=f"sout{c}")
            soutt[c][t] = sout
            cprev = zsout[:, 0, :] if t == 0 else soutt[c][t - 1][:, 0, :]
            nc.vector.tensor_scalar(cand[:, :], u[:, 0:HW], 2.0, -1.0, MULT, ADD)
            nc.vector.tensor_tensor(pt[:, 0, :], u[:, HW:2 * HW], cand[:, :], MULT)
            nc.vector.tensor_tensor(pt[:, 1, :], u[:, 2 * HW:3 * HW], cprev, MULT)
            nc.vector.tensor_tensor(sin[:, 0, :], pt[:, 0, :], pt[:, 1, :], ADD)
            nc.vector.tensor_scalar(sin[:, 1, :], sin[:, 0, :], AL_PHI, BE_PHI,
                                    MULT, ADD)
            nc.scalar.activation(sout[:, :, :], sin[:, :, :], SIG)

        def dve_h(c, t):
            u = ut[c][t]
            sout = soutt[c][t]
            phi3 = sout[:, 1, :].rearrange("p (k b) -> p k b", k=2)
            uo3 = u[:, 3 * HW:4 * HW].rearrange("p (k b) -> p k b", k=2)
            if fp8_r:
                nc.vector.tensor_tensor(h8t[c][:, t, :, :], phi3, uo3, MULT)
            if DBG is None:
                nc.vector.tensor_tensor(hall[c][:, :, t, :], phi3, uo3, MULT)
            else:
                src = {"uc": u[:, 0:HW], "ui": u[:, HW:2 * HW],
                       "uf": u[:, 2 * HW:3 * HW], "uo": u[:, 3 * HW:4 * HW],
                       "cp": sout[:, 0, :], "phi": sout[:, 1, :]}[DBG]
                nc.vector.tensor_copy(
                    hall[c][:, :, t, :],
                    src.rearrange("p (k b) -> p k b", k=2))
            # free per-step psum/u references
            gtl[c][t] = None
            if t >= 1:
                ut[c][t - 1] = None
                soutt[c][t - 1] = None

        def dma_out(c, t):
            t0 = t + 1 - SEG
            for ks in range(2):
                nc.sync.dma_start(
                    out=outd[c, ks, :, t0 * NB:(t + 1) * NB],
                    in_=hall[c][:, ks, t0:t + 1, :],
                )

        # prologue: first x block + projections for steps 0,1
        for c in range(N_CH):
            dma_xblk(c, 0)
        for c in range(N_CH):
            for t in range(2):
                proj(c, t)

        for p in range(NPOS):
            # x DMA 4 positions ahead of first use
            if (p + 6) % XBLK == 0:
                b = (p + 6) // XBLK
                if b < NBLK:
                    for c in range(N_CH):
                        dma_xblk(c, b)
            for c in range(N_CH):
                if p + 2 < NPOS:
                    proj(c, p + 2)
                rstep(c, p)
            for c in range(N_CH):
                dve_pre(c, p)
            for c in range(N_CH):
                dve_h(c, p)
            if (p + 1) % SEG == 0:
                for c in range(N_CH):
                    dma_out(c, p)
    _split_syncs(nc)
    return nc


def _prep_weights(Wx, Rx, bc):
    # reference gate order [i f o c] -> kernel chunk order [c i f o]
    perm = np.concatenate([
        np.arange(3 * U, 4 * U), np.arange(0, U),
        np.arange(U, 2 * U), np.arange(2 * U, 3 * U),
    ])
    Wp = np.ascontiguousarray(Wx[:, perm]).astype(np.float32)
    Rp = np.ascontiguousarray(Rx[:, perm]).astype(np.float32)
    Wp[:, :U] *= 2.0   # cand doubled: tanh(a) = 2*sigmoid(2a)-1
    Rp[:, :U] *= 2.0
    Rp *= K_PHI        # fold K into recurrence
    # [d, g] -> [128, 2(k), g] with k = d-half (DoubleRow k-tiles)
    Wk = np.ascontiguousarray(Wp.reshape(2, 128, G).transpose(1, 0, 2))
    Rk = np.ascontiguousarray(Rp.reshape(2, 128, G).transpose(1, 0, 2))
    bck = (2.0 * np.asarray(bc, np.float32)).reshape(1, 256)
    return Wk, Rk, bck


def kernel(x, W_f, R_f, bc_f, W_b, R_b, bc_b):
    import ml_dtypes
    from concourse.bass_utils import run_bass_kernel_spmd

    FP8NP = ml_dtypes.float8_e4m3
    BF16NP = ml_dtypes.bfloat16
    fp8_proj = _CACHE.get("fp8_proj", True)
    fp8_r = _CACHE.get("fp8_r", True)
    XNP = FP8NP if fp8_proj else BF16NP
    RNP = FP8NP if fp8_r else BF16NP

    x = np.asarray(x, dtype=np.float32)
    if "nc" not in _CACHE:
        _CACHE["nc"] = _build_v4(fp8_proj=fp8_proj, fp8_r=fp8_r)
    nc = _CACHE["nc"]

    Wkf, Rkf, bcf = _prep_weights(np.asarray(W_f, np.float32),
                                  np.asarray(R_f, np.float32),
                                  np.asarray(bc_f, np.float32))
    Wkb, Rkb, bcb = _prep_weights(np.asarray(W_b, np.float32),
                                  np.asarray(R_b, np.float32),
                                  np.asarray(bc_b, np.float32))

    xrev = x[:, ::-1, :]
    in_maps = []
    starts = []  # per core: list of (ch, j, t0, ws)
    for core in range(8):
        fwd = core < 4
        q = core % 4
        xdir = x if fwd else xrev
        xarr = np.empty((N_CH, 2, 128, TL * NB), dtype=XNP)
        st = []
        for ch in range(N_CH):
            for j in range(M_GRP):
                t0 = 512 * q + (512 // N_CH) * ch + L_OUT * j
                ws = max(t0 - W_UP, 0)
                st.append((ch, j, t0, ws))
                # window [ws, ws+TL), lanes j*16..j*16+16
                win = xdir[:, ws:ws + TL, :]          # [B, TL, D]
                wnd = win.transpose(2, 1, 0)          # [D, TL, B]
                wnd = wnd.reshape(2, 128, TL, B).astype(XNP)
                # place lanes j*16:(j+1)*16 for all t
                xv = xarr[ch].reshape(2, 128, TL, NB)
                xv[:, :, :, j * 16:(j + 1) * 16] = wnd
        starts.append(st)
        wk, rk, bck = (Wkf, Rkf, bcf) if fwd else (Wkb, Rkb, bcb)
        in_maps.append({
            "xt": xarr,
            "wd": wk.astype(XNP),
            "rd": rk.astype(RNP),
            "bcd": bck.astype(BF16NP),
        })

    res = run_bass_kernel_spmd(nc, in_maps, core_ids=list(range(8)))
    _CACHE["last_res"] = res.results[0]["outd"]

    outp = np.empty((B, T, 2 * U), dtype=np.float32)
    for core in range(8):
        fwd = core < 4
        od = np.asarray(res.results[core]["outd"])  # [N_CH, 2, 128, TL*NB]
        od = od.reshape(N_CH, 256, TL, M_GRP, 16)
        for (ch, j, t0, ws) in starts[core]:
            off = t0 - ws
            slab = od[ch, :, off:off + L_OUT, j, :]   # [256, L, 16]
            hb = slab.transpose(2, 1, 0).astype(np.float32) * K_PHI
            cs = slice(0, U) if fwd else slice(U, 2 * U)
            outp[:, t0:t0 + L_OUT, cs] = hb
    return outp
